# revision 32
# baseline (speedup 1.0000x reference)
"""Trainium2 Bass kernel for nn_Distribution_74758200754679.

Computes, for x [65536, 8, 256] and a tiny MLP (256 -> 128 -> 1):
    h    = leaky_relu(x @ W1 + b1, 0.3)
    beta = sigmoid(h @ W2 + b2)            # [B, N]
    p    = stick_breaking(beta)            # [B, N+1]

Distribution: pure data parallel over 8 NeuronCores — x is sharded along
the batch axis, MLP params are replicated. Each core's shard is staged
host-side in transposed fp16 layout (d_in on partitions) so the device
loop is a chain of full-rate fp16 matmuls with no on-chip transpose and
half the HBM traffic of fp32.

Per-core device program (32 MB of x per core, 128 blocks x 512 rows):
  DMA xT chunks (ramped sizes so compute starts early)
  -> PE fp16 matmuls (L1, accumulate K=256 in PSUM)
  -> leaky relu in ONE op (alternating ACT Prelu / DVE max(0.3z, z));
     the L2 matmul of block k is emitted after the L1s of block k+1
     (software pipelining) so PE never waits on the leaky op
  -> PE L2 matmul with a [128, 32] stationary (sliding window of a
     [128, 63] tile whose col 31 holds W2): 32 consecutive blocks
     accumulate their beta rows into DISTINCT partitions of one
     [32, 512] PSUM tile
  -> per-group tail (runs under the PE stream): sigmoid straight from
     PSUM + suffix-product stick-breaking + the group's output DMA.
"""

import os
import sys

# The device path runs through jax/PJRT on the neuron (axon) platform; a
# cpu-pinned JAX_PLATFORMS would hide the NeuronCores.
if os.environ.get("JAX_PLATFORMS") == "cpu":
    os.environ["JAX_PLATFORMS"] = ""

for _p in ("/opt/trn_rl_repo",):
    if _p not in sys.path:
        sys.path.insert(0, _p)

import numpy as np
from contextlib import ExitStack

import concourse.bacc as bacc
import concourse.mybir as mybir
from concourse import tile
from concourse import bass_utils

B, N, D_IN, D_H = 65536, 8, 256, 128
SLOPE = 0.3
CORES = 8
RC = B * N // CORES          # rows per core (65536)
BC = B // CORES              # batches per core (8192)
BLK = 512                    # rows per block
NBLK = RC // BLK             # 128
NG = BLK // N                # batch groups per partition in the tail (64)
BGRP = 32                    # blocks per beta PSUM accumulation group
# x DMA chunk sizes in blocks: small first chunks so the PE starts while
# the DMA engines are still ramping
CHUNKS = [1, 1, 2, 4] + [8] * 15
assert sum(CHUNKS) == NBLK

f32 = mybir.dt.float32
f32r = mybir.dt.float32r
f16 = mybir.dt.float16
AF = mybir.ActivationFunctionType
ALU = mybir.AluOpType

_NC_CACHE = {}
_LAST_RESULTS = None


def _build(use_bias: bool):
    nc = bacc.Bacc(
        "TRN2", target_bir_lowering=False, debug=False, num_devices=CORES
    )
    xt_d = nc.dram_tensor("xt", [D_IN, RC], f16, kind="ExternalInput").ap()
    w1_d = nc.dram_tensor("w1", [D_IN, D_H], f16, kind="ExternalInput").ap()
    w2e_d = nc.dram_tensor("w2e", [D_H, BGRP, BGRP], f16, kind="ExternalInput").ap()
    b1c_d = nc.dram_tensor("b1c", [D_H, 1], f32, kind="ExternalInput").ap()
    st_d = nc.dram_tensor("st", [128, 1], f32, kind="ExternalInput").ap()
    nst_d = nc.dram_tensor("nst", [128, 1], f32, kind="ExternalInput").ap()
    p_d = nc.dram_tensor("p", [BC, N + 1], f32, kind="ExternalOutput").ap()

    with tile.TileContext(nc) as tc, ExitStack() as ctx:
        const = ctx.enter_context(tc.tile_pool(name="const", bufs=1))
        xpool = ctx.enter_context(tc.tile_pool(name="xp", bufs=1))
        hpool = ctx.enter_context(tc.tile_pool(name="hp", bufs=1))
        tpool = ctx.enter_context(tc.tile_pool(name="tp", bufs=1))
        psh = ctx.enter_context(tc.tile_pool(name="psh", bufs=1, space="PSUM"))
        psb = ctx.enter_context(tc.tile_pool(name="psb", bufs=1, space="PSUM"))

        def T(pool, shape, dt_, nm, bufs=1):
            tag = nm.split("_")[0]
            return pool.tile(shape, dt_, name=nm, tag=tag, bufs=bufs)

        # w1 + x chunks ride the sync queue; everything else goes through
        # the scalar engine's queue so the first matmul's inputs aren't
        # stuck behind 260 KB of constants during the slow DMA ramp.
        w1_sb = T(const, [128, 2, D_H], f16, "w1sb")
        nc.sync.dma_start(w1_sb[:], w1_d.rearrange("(kc p) m -> p kc m", kc=2))
        w2e_sb = T(const, [D_H, BGRP, BGRP], f16, "w2esb")
        nc.scalar.dma_start(w2e_sb[:], w2e_d[:])
        b1c_sb = T(const, [D_H, 1], f32, "b1csb")
        nc.scalar.dma_start(b1c_sb[:], b1c_d[:])
        st_sb = T(const, [128, 1], f32, "stsb")
        nc.scalar.dma_start(st_sb[:], st_d[:])
        nst_sb = T(const, [128, 1], f32, "nstsb")
        nc.scalar.dma_start(nst_sb[:], nst_d[:])

        def tail_group(g, pbeta_):
            """Stick-breaking for 32 blocks' betas, straight from PSUM.

            Runs under the PE stream for groups 0..2; only group 3's
            chain trails the last matmul.
            """
            sg = T(tpool, [BGRP, BLK], f32, f"sg_{g}")
            nc.scalar.activation(
                sg[:], pbeta_[:], AF.Sigmoid, bias=st_sb[0:BGRP, :], scale=1.0
            )
            gg = T(tpool, [BGRP, BLK], f32, f"gg_{g}")  # 1 - beta
            nc.scalar.activation(
                gg[:], pbeta_[:], AF.Sigmoid, bias=nst_sb[0:BGRP, :], scale=-1.0
            )
            # suffix products s[e] = prod_{k>=e} gg[k] via in-place
            # log-tree: s[0:N-k] *= s[k:N] (forward refs are safe).
            # First level reads gg directly (saves a full copy).
            s = T(tpool, [BGRP, BLK], f32, f"s_{g}")
            sv = s[:].rearrange("p (gr e) -> p gr e", e=N)
            gv = gg[:].rearrange("p (gr e) -> p gr e", e=N)
            nc.vector.tensor_mul(sv[:, :, 0:N - 1], gv[:, :, 0:N - 1], gv[:, :, 1:N])
            nc.vector.tensor_copy(sv[:, :, N - 1:N], gv[:, :, N - 1:N])
            for k in (2, 4):
                nc.vector.tensor_mul(
                    sv[:, :, 0:N - k], sv[:, :, 0:N - k], sv[:, :, k:N]
                )
            # P[gr*9]   = s[gr*8]                    (p[b, 0])
            # P[gr*9+i] = beta[i-1] * s[i], i=1..7;  P[gr*9+8] = beta[7]
            P = T(tpool, [BGRP, NG * (N + 1)], f32, f"P_{g}")
            Pv = P[:].rearrange("p (gr e) -> p gr e", e=N + 1)
            sgv = sg[:].rearrange("p (gr e) -> p gr e", e=N)
            nc.vector.tensor_copy(Pv[:, :, 0:1], sv[:, :, 0:1])
            nc.vector.tensor_mul(Pv[:, :, 1:N], sgv[:, :, 0:N - 1], sv[:, :, 1:N])
            nc.vector.tensor_copy(Pv[:, :, N:N + 1], sgv[:, :, N - 1:N])
            rows = BGRP * NG  # 2048 batches per group
            nc.sync.dma_start(
                p_d[g * rows:(g + 1) * rows, :].rearrange(
                    "(blk gr) e -> blk (gr e)", gr=NG
                ),
                P[:],
            )

        # software pipelining: the L2 matmul of block k is emitted after
        # the L1 matmuls of block k+1, so the PE never waits on the leaky
        # activation of the block it just produced.
        pend = None  # (hh, sub32, pbeta)

        def emit_l2(p):
            hh_, sub32_, pbeta_ = p
            nc.tensor.matmul(
                pbeta_[:], w2e_sb[:, sub32_, :], hh_[:],
                start=(sub32_ == 0), stop=(sub32_ == BGRP - 1),
            )
            if sub32_ == BGRP - 1:
                tail_group(blkcnt[0] // BGRP, pbeta_)
            blkcnt[0] += 1

        blkcnt = [0]  # index of the next L2-emitted block
        pbeta = None

        blk0 = 0
        for ci, cblocks in enumerate(CHUNKS):
            dcols = cblocks * BLK
            c0 = blk0 * BLK
            bufs = 1 if cblocks < 8 else 6
            x0 = T(xpool, [128, dcols], f16, f"x0c{cblocks}_{ci}", bufs=bufs)
            nc.sync.dma_start(x0[:], xt_d[0:128, c0:c0 + dcols])
            x1 = T(xpool, [128, dcols], f16, f"x1c{cblocks}_{ci}", bufs=bufs)
            nc.sync.dma_start(x1[:], xt_d[128:256, c0:c0 + dcols])
            for sub in range(cblocks):
                blk = blk0 + sub
                sub32 = blk % BGRP
                cs = slice(sub * BLK, (sub + 1) * BLK)
                if sub32 == 0:
                    pbeta = T(psb, [BGRP, BLK], f32, f"pbeta_{blk}", bufs=2)

                ph = T(psh, [128, BLK], f32, f"ph_{blk}", bufs=6)
                nc.tensor.matmul(ph[:], w1_sb[:, 0, :], x0[:, cs], start=True, stop=False)
                nc.tensor.matmul(ph[:], w1_sb[:, 1, :], x1[:, cs], start=False, stop=True)
                if pend is not None:
                    emit_l2(pend)
                    pend = None

                hh = T(hpool, [128, BLK], f16, f"hh_{blk}", bufs=4)
                if use_bias or blk % 2 == 1:
                    nc.scalar.activation(
                        hh[:], ph[:], AF.Prelu,
                        bias=b1c_sb[:], scale=1.0, alpha=SLOPE,
                    )
                else:
                    # b1 == 0: leaky_relu(z) = max(0.3*z, z). Only one DVE
                    # input may read PSUM, so stage z in SBUF first.
                    zc = T(hpool, [128, BLK], f16, f"zc_{blk}", bufs=3)
                    nc.vector.tensor_copy(zc[:], ph[:])
                    nc.vector.scalar_tensor_tensor(
                        hh[:], zc[:], SLOPE, zc[:], op0=ALU.mult, op1=ALU.max
                    )
                pend = (hh, sub32, pbeta)
            blk0 += cblocks
        emit_l2(pend)
        pend = None

    nc.compile()
    return nc


def _get_nc(use_bias: bool = False):
    if use_bias not in _NC_CACHE:
        _NC_CACHE[use_bias] = _build(use_bias)
    return _NC_CACHE[use_bias]


def kernel(**inputs):
    x = np.asarray(inputs["x"], dtype=np.float32)
    W1 = np.ascontiguousarray(
        np.asarray(inputs["W1"], dtype=np.float32).astype(np.float16)
    )
    b1 = np.asarray(inputs["b1"], dtype=np.float32)
    W2 = np.ascontiguousarray(np.asarray(inputs["W2"], dtype=np.float32))
    b2 = np.asarray(inputs["b2"], dtype=np.float32)

    use_bias = bool(np.any(b1 != 0.0))
    nc = _get_nc(use_bias)

    xf = x.reshape(B * N, D_IN)
    w2e = np.zeros((D_H, BGRP, BGRP), np.float16)
    for j in range(BGRP):
        w2e[:, j, j] = W2[:, 0].astype(np.float16)
    b1c = np.ascontiguousarray(b1.reshape(D_H, 1))
    stv = np.full((128, 1), np.float32(b2[0]), np.float32)
    nstv = np.ascontiguousarray(-stv)

    in_maps = []
    for c in range(CORES):
        shard = xf[c * RC:(c + 1) * RC]
        xt = shard.T.astype(np.float16)      # [256, RC], fp16 halves HBM traffic
        in_maps.append({
            "xt": xt, "w1": W1, "w2e": w2e,
            "b1c": b1c, "st": stv, "nst": nstv,
        })

    res = bass_utils.run_bass_kernel_spmd(
        nc, in_maps, core_ids=list(range(CORES))
    )
    global _LAST_RESULTS
    _LAST_RESULTS = res
    p = np.concatenate(
        [res.results[c]["p"] for c in range(CORES)], axis=0
    ).astype(np.float32)
    return p


# revision 33
# speedup vs baseline: 1.0076x; 1.0076x over previous
"""Trainium2 Bass kernel for nn_Distribution_74758200754679.

Computes, for x [65536, 8, 256] and a tiny MLP (256 -> 128 -> 1):
    h    = leaky_relu(x @ W1 + b1, 0.3)
    beta = sigmoid(h @ W2 + b2)            # [B, N]
    p    = stick_breaking(beta)            # [B, N+1]

Distribution: pure data parallel over 8 NeuronCores — x is sharded along
the batch axis, MLP params are replicated. Each core's shard is staged
host-side in transposed fp16 layout (d_in on partitions) so the device
loop is a chain of full-rate fp16 matmuls with no on-chip transpose and
half the HBM traffic of fp32.

Per-core device program (32 MB of x per core, 128 blocks x 512 rows):
  DMA xT chunks (ramped sizes so compute starts early)
  -> PE fp16 matmuls (L1, accumulate K=256 in PSUM)
  -> leaky relu in ONE op (alternating ACT Prelu / DVE max(0.3z, z));
     the L2 matmul of block k is emitted after the L1s of block k+1
     (software pipelining) so PE never waits on the leaky op
  -> PE L2 matmul with a [128, 32] stationary (sliding window of a
     [128, 63] tile whose col 31 holds W2): 32 consecutive blocks
     accumulate their beta rows into DISTINCT partitions of one
     [32, 512] PSUM tile
  -> per-group tail (runs under the PE stream): sigmoid straight from
     PSUM + suffix-product stick-breaking + the group's output DMA.
"""

import os
import sys

# The device path runs through jax/PJRT on the neuron (axon) platform; a
# cpu-pinned JAX_PLATFORMS would hide the NeuronCores.
if os.environ.get("JAX_PLATFORMS") == "cpu":
    os.environ["JAX_PLATFORMS"] = ""

for _p in ("/opt/trn_rl_repo",):
    if _p not in sys.path:
        sys.path.insert(0, _p)

import numpy as np
from contextlib import ExitStack

import concourse.bacc as bacc
import concourse.mybir as mybir
from concourse import tile
from concourse import bass_utils

B, N, D_IN, D_H = 65536, 8, 256, 128
SLOPE = 0.3
CORES = 8
RC = B * N // CORES          # rows per core (65536)
BC = B // CORES              # batches per core (8192)
BLK = 512                    # rows per block
NBLK = RC // BLK             # 128
NG = BLK // N                # batch groups per partition in the tail (64)
BGRP = 32                    # blocks per beta PSUM accumulation group
# x DMA chunk sizes in blocks: small first chunks so the PE starts while
# the DMA engines are still ramping
CHUNKS = [1, 1, 2, 4] + [8] * 15
assert sum(CHUNKS) == NBLK

f32 = mybir.dt.float32
f32r = mybir.dt.float32r
f16 = mybir.dt.float16
AF = mybir.ActivationFunctionType
ALU = mybir.AluOpType

_NC_CACHE = {}
_LAST_RESULTS = None


def _build(use_bias: bool):
    nc = bacc.Bacc(
        "TRN2", target_bir_lowering=False, debug=False, num_devices=CORES
    )
    xt_d = nc.dram_tensor("xt", [D_IN, RC], f16, kind="ExternalInput").ap()
    w1_d = nc.dram_tensor("w1", [D_IN, D_H], f16, kind="ExternalInput").ap()
    w2e_d = nc.dram_tensor("w2e", [D_H, BGRP, BGRP], f16, kind="ExternalInput").ap()
    b1c_d = nc.dram_tensor("b1c", [D_H, 1], f32, kind="ExternalInput").ap()
    st_d = nc.dram_tensor("st", [128, 1], f32, kind="ExternalInput").ap()
    nst_d = nc.dram_tensor("nst", [128, 1], f32, kind="ExternalInput").ap()
    p_d = nc.dram_tensor("p", [BC, N + 1], f32, kind="ExternalOutput").ap()

    with tile.TileContext(nc) as tc, ExitStack() as ctx:
        const = ctx.enter_context(tc.tile_pool(name="const", bufs=1))
        xpool = ctx.enter_context(tc.tile_pool(name="xp", bufs=1))
        hpool = ctx.enter_context(tc.tile_pool(name="hp", bufs=1))
        tpool = ctx.enter_context(tc.tile_pool(name="tp", bufs=1))
        psh = ctx.enter_context(tc.tile_pool(name="psh", bufs=1, space="PSUM"))
        psb = ctx.enter_context(tc.tile_pool(name="psb", bufs=1, space="PSUM"))

        def T(pool, shape, dt_, nm, bufs=1):
            tag = nm.split("_")[0]
            return pool.tile(shape, dt_, name=nm, tag=tag, bufs=bufs)

        # w1 + x chunks ride the sync queue; everything else goes through
        # the scalar engine's queue so the first matmul's inputs aren't
        # stuck behind 260 KB of constants during the slow DMA ramp.
        w1_sb = T(const, [128, 2, D_H], f16, "w1sb")
        nc.sync.dma_start(w1_sb[:], w1_d.rearrange("(kc p) m -> p kc m", kc=2))
        w2e_sb = T(const, [D_H, BGRP, BGRP], f16, "w2esb")
        nc.scalar.dma_start(w2e_sb[:], w2e_d[:])
        b1c_sb = T(const, [D_H, 1], f32, "b1csb")
        nc.scalar.dma_start(b1c_sb[:], b1c_d[:])
        st_sb = T(const, [128, 1], f32, "stsb")
        nc.scalar.dma_start(st_sb[:], st_d[:])
        nst_sb = T(const, [128, 1], f32, "nstsb")
        nc.scalar.dma_start(nst_sb[:], nst_d[:])

        def tail_group(g, pbeta_):
            """Stick-breaking for 32 blocks' betas, straight from PSUM.

            Runs under the PE stream for groups 0..2; only group 3's
            chain trails the last matmul.
            """
            sg = T(tpool, [BGRP, BLK], f32, f"sg_{g}")
            nc.scalar.activation(
                sg[:], pbeta_[:], AF.Sigmoid, bias=st_sb[0:BGRP, :], scale=1.0
            )
            gg = T(tpool, [BGRP, BLK], f32, f"gg_{g}")  # 1 - beta
            nc.scalar.activation(
                gg[:], pbeta_[:], AF.Sigmoid, bias=nst_sb[0:BGRP, :], scale=-1.0
            )
            # suffix products s[e] = prod_{k>=e} gg[k] via in-place
            # log-tree: s[0:N-k] *= s[k:N] (forward refs are safe).
            # First level reads gg directly (saves a full copy).
            s = T(tpool, [BGRP, BLK], f32, f"s_{g}")
            sv = s[:].rearrange("p (gr e) -> p gr e", e=N)
            gv = gg[:].rearrange("p (gr e) -> p gr e", e=N)
            nc.vector.tensor_mul(sv[:, :, 0:N - 1], gv[:, :, 0:N - 1], gv[:, :, 1:N])
            nc.vector.tensor_copy(sv[:, :, N - 1:N], gv[:, :, N - 1:N])
            for k in (2, 4):
                nc.vector.tensor_mul(
                    sv[:, :, 0:N - k], sv[:, :, 0:N - k], sv[:, :, k:N]
                )
            # P[gr*9]   = s[gr*8]                    (p[b, 0])
            # P[gr*9+i] = beta[i-1] * s[i], i=1..7;  P[gr*9+8] = beta[7]
            P = T(tpool, [BGRP, NG * (N + 1)], f32, f"P_{g}")
            Pv = P[:].rearrange("p (gr e) -> p gr e", e=N + 1)
            sgv = sg[:].rearrange("p (gr e) -> p gr e", e=N)
            nc.vector.tensor_copy(Pv[:, :, 0:1], sv[:, :, 0:1])
            nc.vector.tensor_mul(Pv[:, :, 1:N], sgv[:, :, 0:N - 1], sv[:, :, 1:N])
            nc.vector.tensor_copy(Pv[:, :, N:N + 1], sgv[:, :, N - 1:N])
            rows = BGRP * NG  # 2048 batches per group
            nc.gpsimd.dma_start(
                p_d[g * rows:(g + 1) * rows, :].rearrange(
                    "(blk gr) e -> blk (gr e)", gr=NG
                ),
                P[:],
            )

        # software pipelining: the L2 matmul of block k is emitted after
        # the L1 matmuls of block k+1, so the PE never waits on the leaky
        # activation of the block it just produced.
        pend = None  # (hh, sub32, pbeta)

        def emit_l2(p):
            hh_, sub32_, pbeta_ = p
            nc.tensor.matmul(
                pbeta_[:], w2e_sb[:, sub32_, :], hh_[:],
                start=(sub32_ == 0), stop=(sub32_ == BGRP - 1),
            )
            if sub32_ == BGRP - 1:
                tail_group(blkcnt[0] // BGRP, pbeta_)
            blkcnt[0] += 1

        blkcnt = [0]  # index of the next L2-emitted block
        pbeta = None

        blk0 = 0
        for ci, cblocks in enumerate(CHUNKS):
            dcols = cblocks * BLK
            c0 = blk0 * BLK
            bufs = 1 if cblocks < 8 else 6
            x0 = T(xpool, [128, dcols], f16, f"x0c{cblocks}_{ci}", bufs=bufs)
            nc.sync.dma_start(x0[:], xt_d[0:128, c0:c0 + dcols])
            x1 = T(xpool, [128, dcols], f16, f"x1c{cblocks}_{ci}", bufs=bufs)
            nc.sync.dma_start(x1[:], xt_d[128:256, c0:c0 + dcols])
            for sub in range(cblocks):
                blk = blk0 + sub
                sub32 = blk % BGRP
                cs = slice(sub * BLK, (sub + 1) * BLK)
                if sub32 == 0:
                    pbeta = T(psb, [BGRP, BLK], f32, f"pbeta_{blk}", bufs=2)

                ph = T(psh, [128, BLK], f32, f"ph_{blk}", bufs=6)
                nc.tensor.matmul(ph[:], w1_sb[:, 0, :], x0[:, cs], start=True, stop=False)
                nc.tensor.matmul(ph[:], w1_sb[:, 1, :], x1[:, cs], start=False, stop=True)
                if pend is not None:
                    emit_l2(pend)
                    pend = None

                hh = T(hpool, [128, BLK], f16, f"hh_{blk}", bufs=4)
                if use_bias or blk % 2 == 1:
                    nc.scalar.activation(
                        hh[:], ph[:], AF.Prelu,
                        bias=b1c_sb[:], scale=1.0, alpha=SLOPE,
                    )
                else:
                    # b1 == 0: leaky_relu(z) = max(0.3*z, z). Only one DVE
                    # input may read PSUM, so stage z in SBUF first.
                    zc = T(hpool, [128, BLK], f16, f"zc_{blk}", bufs=3)
                    nc.vector.tensor_copy(zc[:], ph[:])
                    nc.vector.scalar_tensor_tensor(
                        hh[:], zc[:], SLOPE, zc[:], op0=ALU.mult, op1=ALU.max
                    )
                pend = (hh, sub32, pbeta)
            blk0 += cblocks
        emit_l2(pend)
        pend = None

    nc.compile()
    return nc


def _get_nc(use_bias: bool = False):
    if use_bias not in _NC_CACHE:
        _NC_CACHE[use_bias] = _build(use_bias)
    return _NC_CACHE[use_bias]


def kernel(**inputs):
    x = np.asarray(inputs["x"], dtype=np.float32)
    W1 = np.ascontiguousarray(
        np.asarray(inputs["W1"], dtype=np.float32).astype(np.float16)
    )
    b1 = np.asarray(inputs["b1"], dtype=np.float32)
    W2 = np.ascontiguousarray(np.asarray(inputs["W2"], dtype=np.float32))
    b2 = np.asarray(inputs["b2"], dtype=np.float32)

    use_bias = bool(np.any(b1 != 0.0))
    nc = _get_nc(use_bias)

    xf = x.reshape(B * N, D_IN)
    w2e = np.zeros((D_H, BGRP, BGRP), np.float16)
    for j in range(BGRP):
        w2e[:, j, j] = W2[:, 0].astype(np.float16)
    b1c = np.ascontiguousarray(b1.reshape(D_H, 1))
    stv = np.full((128, 1), np.float32(b2[0]), np.float32)
    nstv = np.ascontiguousarray(-stv)

    in_maps = []
    for c in range(CORES):
        shard = xf[c * RC:(c + 1) * RC]
        xt = shard.T.astype(np.float16)      # [256, RC], fp16 halves HBM traffic
        in_maps.append({
            "xt": xt, "w1": W1, "w2e": w2e,
            "b1c": b1c, "st": stv, "nst": nstv,
        })

    res = bass_utils.run_bass_kernel_spmd(
        nc, in_maps, core_ids=list(range(CORES))
    )
    global _LAST_RESULTS
    _LAST_RESULTS = res
    p = np.concatenate(
        [res.results[c]["p"] for c in range(CORES)], axis=0
    ).astype(np.float32)
    return p
